# revision 13
# baseline (speedup 1.0000x reference)
"""Causal self-attention (B=4, S=2048, D=1024, H=16) on 8 Trainium2 NeuronCores.

Sharding: 8 cores = 4 batches x 2 head-groups (8 heads each).
Per core: QKV projections, flash-style causal attention with scores computed
transposed ([k, q] layout), exp on ScalarE (no max subtraction -- scores are
O(1)), softmax denominator via an appended ones-column in the attn@V matmul,
out-projection against a W_O column slice.  The inter-core "all-reduce" for
the out-projection (row-parallel W_O) is a host-side sum of the two
head-group partials per batch.

v2 restructure vs the original baseline:
  - x is DMA'd in 512-column blocks so V/QK projections start ~3-6us in
    instead of waiting for the full 4MB transfer.
  - Attention is emitted per-kc (one [128, 2, 512] score tile = both heads of
    one key chunk) with a double-buffered score PSUM ring, so the
    scores->exp->attn@V chain pipelines without PSUM stalls.
  - Attention blocks are spread across the whole kernel (block (qb,hp) is
    emitted as soon as qk-proj(hp) columns <= qb and V chunks <= 4qb+3 are
    done), with remaining projection / out-projection tiles interleaved as
    fillers so the tensor engine never idles waiting on ScalarE's exp.
  - Diagonal key-chunks are trimmed: scores/exp/mask/attn@V only touch query
    columns >= 128*jj that the causal mask can keep.
  - exp covers both heads in one ACTIVATE (A in [:,0,:], B in [:,1,:]).
  - ACT exp table is preloaded during the input DMA.
"""

import os
import sys

for _p in ("/opt/trn_rl_repo",):
    if _p not in sys.path and os.path.isdir(_p):
        sys.path.insert(0, _p)

import numpy as np

B, S, D, H, DK = 4, 2048, 1024, 16, 64
N_CORES = 8
EC = 512          # e-dims (= head-dim columns) per core: 8 heads x 64
N_D = D // 128    # 8 contraction chunks for projections
N_SC = S // 128   # 16 key chunks
N_QB = S // 512   # 4 query blocks

_CACHE = {}


def _build():
    import concourse.mybir as mybir
    import concourse.tile as tile
    from concourse import bacc
    from contextlib import ExitStack

    fp32 = mybir.dt.float32
    bf16 = mybir.dt.bfloat16
    AF = mybir.ActivationFunctionType
    Alu = mybir.AluOpType

    nc = bacc.Bacc(trn_type="TRN2", target_bir_lowering=False, debug=False)

    xt_d = nc.dram_tensor("xt", [D, S], bf16, kind="ExternalInput")
    wq_d = nc.dram_tensor("wqt", [D, EC], bf16, kind="ExternalInput")
    wk_d = nc.dram_tensor("wkt", [D, EC], bf16, kind="ExternalInput")
    wv_d = nc.dram_tensor("wvt", [D, EC], bf16, kind="ExternalInput")
    wo_d = nc.dram_tensor("wot", [EC, D], bf16, kind="ExternalInput")
    yt_d = nc.dram_tensor("yt", [D, S], fp32, kind="ExternalOutput")

    with tile.TileContext(nc) as tc, ExitStack() as ctx:
        # ---- persistent projection outputs ------------------------------
        proj_out_pool = ctx.enter_context(tc.tile_pool(name="projout", bufs=1))
        qt_sb = [proj_out_pool.tile([128, S], bf16, name=f"qt{ec}", tag=f"qt{ec}") for ec in range(4)]
        kt_sb = [proj_out_pool.tile([128, S], bf16, name=f"kt{ec}", tag=f"kt{ec}") for ec in range(4)]
        # v_sb[sc]: per head h a 128-col stationary block:
        #   even h: [V(64) | ones(col 64) | unused(63)] -> psum rows 0..64
        #   odd  h: [zeros(0:63), ones at col 32 | V(64) at 64:128]
        #           -> psum row 32 = n, rows 64..127 = out
        v_sb = [proj_out_pool.tile([128, 8, 128], bf16, name=f"v{sc}", tag=f"v{sc}") for sc in range(N_SC)]

        # ---- input tiles (all bf16) -------------------------------------
        xw_pool = ctx.enter_context(tc.tile_pool(name="xw", bufs=1))
        xt_sb = [xw_pool.tile([128, S], bf16, name=f"x{d}", tag=f"x{d}") for d in range(N_D)]
        wv_sb = [xw_pool.tile([128, EC], bf16, name=f"wv{d}", tag=f"wv{d}") for d in range(N_D)]
        wq_sb = [xw_pool.tile([128, EC], bf16, name=f"wq{d}", tag=f"wq{d}") for d in range(N_D)]
        wk_sb = [xw_pool.tile([128, EC], bf16, name=f"wk{d}", tag=f"wk{d}") for d in range(N_D)]
        wo_sb = [xw_pool.tile([128, D], bf16, name=f"wo{cc}", tag=f"wo{cc}") for cc in range(4)]

        # DMA issue cost (~0.6us each) serializes per queue, so spread the
        # input across three queues: wk + masks on gpsimd (SWDGE) so the
        # first score tiles and masks are ready early, x on the scalar
        # queue (HWDGE), wv/wq/wo on sync (HWDGE).
        const_pool = ctx.enter_context(tc.tile_pool(name="const", bufs=1))
        # preload the exp table set during the DMA wait (scalar queue FIRST,
        # before anything else lands on it)
        warm = const_pool.tile([128, 16], fp32, name="warm", tag="warm")
        nc.vector.memset(warm[:], 0.0)
        nc.scalar.activation(warm[:], warm[:], AF.Exp, scale=0.125)

        for d in range(N_D):
            nc.gpsimd.dma_start(wk_sb[d][:], wk_d.ap()[128 * d:128 * (d + 1), :])
        for d in range(N_D):
            nc.sync.dma_start(wv_sb[d][:], wv_d.ap()[128 * d:128 * (d + 1), :])
        for d in range(N_D):
            nc.scalar.dma_start(
                xt_sb[d][:, 0:512], xt_d.ap()[128 * d:128 * (d + 1), 0:512])
        for d in range(N_D):
            nc.sync.dma_start(wq_sb[d][:], wq_d.ap()[128 * d:128 * (d + 1), :])
        for d in range(N_D):
            nc.scalar.dma_start(
                xt_sb[d][:, 512:1024], xt_d.ap()[128 * d:128 * (d + 1), 512:1024])
        for cc in range(4):
            nc.sync.dma_start(wo_sb[cc][:], wo_d.ap()[128 * cc:128 * (cc + 1), :])
        for d in range(N_D):
            nc.scalar.dma_start(
                xt_sb[d][:, 1024:2048], xt_d.ap()[128 * d:128 * (d + 1), 1024:2048])

        # ---- constant masks for the 4 diagonal positions (gpsimd, right
        # after the wk issues so they are ready before the first block) ----
        masks = []
        for j in range(4):
            m = const_pool.tile([128, 512], bf16, name=f"mask{j}", tag=f"mask{j}")
            nc.gpsimd.memset(m[:], 1.0)
            # keep 1.0 where q_rel >= p + 128*j  (q >= k), else 0
            nc.gpsimd.affine_select(
                out=m[:], in_=m[:], compare_op=Alu.is_ge, fill=0.0,
                base=-128 * j, pattern=[[1, 512]], channel_multiplier=-1,
            )
            masks.append(m)

        # ---- pools ------------------------------------------------------
        dram_pool = ctx.enter_context(tc.tile_pool(name="drs", bufs=4, space="DRAM"))
        ps_work = ctx.enter_context(tc.tile_pool(name="pswork", bufs=2, space="PSUM"))
        ps_score = tc.alloc_tile_pool(name="psscore", bufs=2, space="PSUM")
        ps_av = tc.alloc_tile_pool(name="psav", bufs=1, space="PSUM")
        attn_pool = ctx.enter_context(tc.tile_pool(name="attn", bufs=4))
        rb_pool = ctx.enter_context(tc.tile_pool(name="rb", bufs=2))
        ou_pool = ctx.enter_context(tc.tile_pool(name="ou", bufs=2))
        outn_pool = ctx.enter_context(tc.tile_pool(name="outn", bufs=4))
        y_pool = ctx.enter_context(tc.tile_pool(name="ysb", bufs=3))

        outn_all = {qb: [outn_pool.tile([128, 512], bf16, name=f"on{qb}{hp}", tag=f"on{hp}")
                         for hp in range(4)] for qb in range(N_QB)}

        # ---- emission units ---------------------------------------------
        def emit_v_proj(sc):
            ps = ps_work.tile([128, 512], fp32, name="pv", tag="pp")
            for d in range(N_D):
                nc.tensor.matmul(
                    ps[:],
                    xt_sb[d][:, 128 * sc:128 * (sc + 1)],
                    wv_sb[d][:],
                    start=(d == 0), stop=(d == N_D - 1),
                )
            vt = v_sb[sc]
            for h in range(8):
                if h % 2 == 0:
                    nc.vector.tensor_copy(vt[:, h, 0:64], ps[:, 64 * h:64 * h + 64])
                    nc.gpsimd.memset(vt[:, h, 64:65], 1.0)
                else:
                    nc.gpsimd.memset(vt[:, h, 0:63], 0.0)
                    nc.gpsimd.memset(vt[:, h, 32:33], 1.0)
                    nc.vector.tensor_copy(vt[:, h, 64:128], ps[:, 64 * h:64 * h + 64])

        def emit_qk_tile(ec, sb_, kind):
            w_sb, out_sb = (wq_sb, qt_sb) if kind == 0 else (wk_sb, kt_sb)
            ps = ps_work.tile([128, 512], fp32, name="pp", tag="pp")
            for d in range(N_D):
                nc.tensor.matmul(
                    ps[:],
                    w_sb[d][:, 128 * ec:128 * (ec + 1)],
                    xt_sb[d][:, 512 * sb_:512 * (sb_ + 1)],
                    start=(d == 0), stop=(d == N_D - 1),
                )
            nc.vector.tensor_copy(out_sb[ec][:, 512 * sb_:512 * (sb_ + 1)], ps[:])

        def emit_outproj_tile(qb, dc):
            outn = outn_all[qb]
            ps = ps_work.tile([128, 512], fp32, name="py", tag="pp")
            for hp in range(4):
                nc.tensor.matmul(
                    ps[:],
                    wo_sb[hp][:, 128 * dc:128 * (dc + 1)],
                    outn[hp][:],
                    start=(hp == 0), stop=(hp == 3),
                )
            ysb = y_pool.tile([128, 512], fp32, name="y", tag="y")
            nc.vector.tensor_copy(ysb[:], ps[:])
            eng = nc.sync if dc % 2 == 0 else nc.gpsimd
            eng.dma_start(
                yt_d.ap()[128 * dc:128 * (dc + 1), 512 * qb:512 * (qb + 1)],
                ysb[:])

        # ---- attention block (qb, hp), per-kc pipelined ------------------
        def emit_attn_block(qb, hp, fillers):
            nkc = 4 * qb + 4
            hA, hB = 2 * hp, 2 * hp + 1
            qt, kt = qt_sb[hp], kt_sb[hp]
            poA = ps_av.tile([128, 512], fp32, name="poA", tag="poA")
            poB = ps_av.tile([128, 512], fp32, name="poB", tag="poB")

            def av(at, kc, lo):
                nc.tensor.matmul(
                    poA[0:65, lo:512], v_sb[kc][:, hA, 0:65], at[:, 0, lo:512],
                    start=(kc == 0), stop=(kc == nkc - 1))
                nc.tensor.matmul(
                    poB[0:128, lo:512], v_sb[kc][:, hB, 0:128], at[:, 1, lo:512],
                    start=(kc == 0), stop=(kc == nkc - 1))

            # spread fillers over the kt_i steps, never at step 0 (a filler
            # emitted too early would stall the tensor queue on its deps)
            # fillers go in pairs (halves the 64<->128-row-mode boundaries),
            # spread over the kt_i steps, never at step 0
            nkt = nkc // 2
            fill_at = {}
            if fillers:
                npair = (len(fillers) + 1) // 2
                for i, f in enumerate(fillers):
                    j = i // 2
                    pos = max(1, int((j + 1) * nkt / (npair + 1))) if nkt > 1 else 0
                    fill_at.setdefault(pos, []).append(f)

            def one_kc(kc):
                jj = kc - 4 * qb
                lo = 128 * jj if jj > 0 else 0
                q0 = 512 * qb
                ps_s = ps_score.tile([128, 2, 512], fp32, name="ps", tag="ps")
                nc.tensor.matmul(
                    ps_s[:, 0, lo:512],
                    kt[0:64, 128 * kc:128 * (kc + 1)],
                    qt[0:64, q0 + lo:q0 + 512],
                    start=True, stop=True)
                nc.tensor.matmul(
                    ps_s[:, 1, lo:512],
                    kt[64:128, 128 * kc:128 * (kc + 1)],
                    qt[64:128, q0 + lo:q0 + 512],
                    start=True, stop=True)
                return ps_s, lo

            pend = None
            for kt_i in range(nkt):
                kcs = (2 * kt_i, 2 * kt_i + 1)
                # both key-chunks' score pairs back-to-back: one 64-row-mode
                # group on the PE array (mode switches drain the array)
                pss = [one_kc(kc) for kc in kcs]
                cur = []
                for kc, (ps_s, lo) in zip(kcs, pss):
                    jj = kc - 4 * qb
                    at = attn_pool.tile([128, 2, 512], bf16, name="at", tag="at")
                    nc.scalar.activation(at[:, :, lo:512], ps_s[:, :, lo:512],
                                         AF.Exp, scale=0.125)
                    if jj >= 0:
                        nc.vector.tensor_tensor(
                            at[:, :, lo:512], at[:, :, lo:512],
                            masks[jj][:, None, lo:512].to_broadcast((128, 2, 512 - lo)),
                            Alu.mult)
                    cur.append((at, kc, lo))
                if pend is not None:
                    for p in pend:
                        av(*p)
                pend = cur
                for f in fill_at.get(kt_i, ()):
                    f()
            for p in pend:
                av(*p)

            # normalization: copy out + n rows off PSUM (frees banks), DRAM
            # round-trip broadcast of the two n rows, one full-tile fast
            # reciprocal (base-0 only), muls into the bf16 outn tiles.
            ou = ou_pool.tile([128, 512], fp32, name="ou", tag="ou")
            rbn = rb_pool.tile([128, 512], fp32, name="rbn", tag="rbn")
            rbi = rb_pool.tile([128, 512], fp32, name="rbi", tag="rbi")
            rbb = rb_pool.tile([128, 512], fp32, name="rbb", tag="rbb")
            nc.vector.tensor_copy(ou[0:64, :], poA[0:64, :])
            nc.vector.tensor_copy(rbn[64:65, :], poA[64:65, :])
            nc.vector.tensor_copy(ou[64:128, :], poB[64:128, :])
            nc.vector.tensor_copy(rbn[32:33, :], poB[32:33, :])
            rdA = dram_pool.tile([1, 512], fp32, name="rdA", tag="rdA")
            rdB = dram_pool.tile([1, 512], fp32, name="rdB", tag="rdB")
            # the two round-trip legs ride different queues so their
            # completion latencies overlap
            nc.sync.dma_start(rdA[:], rbn[64:65, :])
            nc.gpsimd.dma_start(rdB[:], rbn[32:33, :])
            nc.sync.dma_start(rbb[0:64, :], rdA[0:1, :].to_broadcast((64, 512)))
            nc.gpsimd.dma_start(rbb[64:128, :], rdB[0:1, :].to_broadcast((64, 512)))
            nc.vector.reciprocal_approx_fast(out=rbi[:, :], in_=rbb[:, :])
            outn = outn_all[qb]
            nc.vector.tensor_mul(outn[hp][0:64, :], ou[0:64, :], rbi[0:64, :])
            nc.vector.tensor_mul(outn[hp][64:128, :], ou[64:128, :], rbi[64:128, :])

        # ---- global schedule --------------------------------------------
        F = []  # convenience for building filler closures

        def fv(sc):
            return lambda: emit_v_proj(sc)

        def fqk(ec, sb_, kind):
            return lambda: emit_qk_tile(ec, sb_, kind)

        def fop(qb, dc):
            return lambda: emit_outproj_tile(qb, dc)

        # HAM warmup: junk matmuls on the first wk chunks (earliest DMA to
        # land) keep the PE activity window busy so the real projections run
        # at 2.4GHz instead of the cold 1.2GHz rate.  Results are never read
        # (the score ring resets with start=True).
        for i in range(12):
            pw = ps_score.tile([128, 2, 512], fp32, name="pwu", tag="ps")
            nc.tensor.matmul(pw[:, 0, :], wk_sb[0][:, 0:128], wk_sb[1][:, 0:512],
                             start=True, stop=True)

        # prologue: V chunks 0..3 and qk(0) @ sb0 (gated only by early DMA);
        # the K tile first (wk lands before wq)
        for sc in range(4):
            emit_v_proj(sc)
        emit_qk_tile(0, 0, 1)
        emit_qk_tile(0, 0, 0)

        qk = lambda ec, sb_: [fqk(ec, sb_, 0), fqk(ec, sb_, 1)]
        vs = lambda a, b: [fv(sc) for sc in range(a, b)]
        op = lambda qb, a, b: [fop(qb, dc) for dc in range(a, b)]

        # Every filler tile is emitted at least one block before any consumer:
        # block (qb,hp) needs qk(hp) @ sb<=qb and V chunks <= 4qb+3.
        sched = [
            # hp = 0 stretch: finish V + qk(0), start qk(1)
            (0, 0, qk(0, 1) + vs(4, 8)),
            (1, 0, qk(0, 2) + qk(0, 3) + vs(8, 12)),
            (2, 0, qk(1, 0) + qk(1, 1) + vs(12, 16)),
            (3, 0, qk(1, 2) + qk(1, 3)),
            # hp = 1 stretch: start qk(2)
            (0, 1, qk(2, 0)),
            (1, 1, qk(2, 1)),
            (2, 1, qk(2, 2)),
            (3, 1, qk(2, 3)),
            # hp = 2 stretch: finish qk(3)
            (0, 2, qk(3, 0)),
            (1, 2, qk(3, 1)),
            (2, 2, qk(3, 2)),
            (3, 2, qk(3, 3)),
            # hp = 3 stretch: out-projections as fillers; (3,3) runs after
            # (0,3) so op(0) tiles can fill its 16-kc ACT-paced stretch
            (0, 3, []),
            (3, 3, op(0, 0, 8)),
            (1, 3, op(3, 0, 8)),
            (2, 3, op(1, 0, 4)),
        ]
        for qb, hp, fillers in sched:
            emit_attn_block(qb, hp, fillers)
        # tail: held-back op(1) tiles fill the last block's normalization
        # latency, then op(2) on a deeper psum ring (attention pools done --
        # their banks are free)
        ps_av.release()
        ps_score.release()
        ps_tail = tc.alloc_tile_pool(name="pstail", bufs=4, space="PSUM")
        for qb, dc in [(1, 4), (1, 5), (1, 6), (1, 7)] + [(2, dc) for dc in range(8)]:
            outn = outn_all[qb]
            ps = ps_tail.tile([128, 512], fp32, name="pt", tag="pt")
            for hp in range(4):
                nc.tensor.matmul(
                    ps[:],
                    wo_sb[hp][:, 128 * dc:128 * (dc + 1)],
                    outn[hp][:],
                    start=(hp == 0), stop=(hp == 3),
                )
            ysb = y_pool.tile([128, 512], fp32, name="y", tag="y")
            nc.vector.tensor_copy(ysb[:], ps[:])
            eng = nc.sync if dc % 2 == 0 else nc.gpsimd
            eng.dma_start(
                yt_d.ap()[128 * dc:128 * (dc + 1), 512 * qb:512 * (qb + 1)],
                ysb[:])
        ps_tail.release()

    nc.compile()
    return nc


def _get_nc():
    if "nc" not in _CACHE:
        _CACHE["nc"] = _build()
    return _CACHE["nc"]


def _run(in_maps, trace=False, **kw):
    from concourse.bass_utils import run_bass_kernel_spmd
    nc = _get_nc()
    return run_bass_kernel_spmd(nc, in_maps, core_ids=list(range(N_CORES)),
                                trace=trace, **kw)


def _prep_inputs(x, W_Q, W_K, W_V, W_O):
    import ml_dtypes
    bf = ml_dtypes.bfloat16
    x = np.asarray(x, dtype=np.float32)
    W_Q = np.asarray(W_Q, dtype=np.float32)
    W_K = np.asarray(W_K, dtype=np.float32)
    W_V = np.asarray(W_V, dtype=np.float32)
    W_O = np.asarray(W_O, dtype=np.float32)
    in_maps = []
    for c in range(N_CORES):
        b, hg = divmod(c, 2)
        es = EC * hg
        in_maps.append({
            "xt": np.ascontiguousarray(x[b].T).astype(bf),
            "wqt": np.ascontiguousarray(W_Q[es:es + EC, :].T).astype(bf),
            "wkt": np.ascontiguousarray(W_K[es:es + EC, :].T).astype(bf),
            "wvt": np.ascontiguousarray(W_V[es:es + EC, :].T).astype(bf),
            "wot": np.ascontiguousarray(W_O[:, es:es + EC].T).astype(bf),
        })
    return in_maps


def _gather(results):
    y = np.empty((B, S, D), dtype=np.float32)
    for b in range(B):
        yt = results[2 * b]["yt"].astype(np.float32) + results[2 * b + 1]["yt"].astype(np.float32)
        y[b] = yt.T
    return y


def kernel(x, W_Q, W_K, W_V, W_O):
    in_maps = _prep_inputs(x, W_Q, W_K, W_V, W_O)
    res = _run(in_maps, trace=False)
    return _gather(res.results)
